# revision 31
# baseline (speedup 1.0000x reference)
"""FBGAT layer kernel for 8 Trainium2 NeuronCores (v2: fp8 DoubleRow).

Full inputs in, full output out. Row-shards nodes across 8 cores.

High-pass path Hh = d_inv @ lap @ d_inv @ relu(X Wh^T).  With the host
centering d_inv = 0.5*J + Dc and host-precomputed (input-only)
lsum = lap @ 1, v = d_inv @ lsum, and device c = 0.5*colsum(XW0):
    T1 = d_inv @ XW0 = 1 (x) c + T1c,   T1c = Dc @ XW0
    T2 = lap @ T1    = lsum (x) c + P,  P   = lap @ T1c
    Hh = d_inv @ T2  = v (x) c + 1 (x) beta + Dc @ P,
                                        beta = 0.5*colsum(P)
T1c / Ps = P/16 ship between cores as fp8 AllGathers; every chain
matmul is fp8 DoubleRow (two 128-k-planes per instruction).  Rank-1
terms are rebuilt in the output combine.  Total error ~1e-3 of output
absmax (tolerance 2e-2).

GAT: dense [src, dst] form.  Per-dst softmax rescale by exp(-a_dst)
turns exp(leakyrelu(a_src+a_dst)) into max(u1[s], u2[s]*w[d]) with
u1 = exp(a_src), u2 = exp(0.2 a_src), w = exp(-0.8 a_dst) -- the
rescale cancels between numerator and denominator.  So the dense part
is one relu/max pass plus one multiply by the edge-multiplicity
matrix; no dense exp.  Numerator and denominator both come from one PE
matmul with a ones-augmented h.  a_src / a_dst come out of the main
XW matmul via 8 extra weight columns folded host-side.
"""
import os
import sys

sys.path.insert(0, "/opt/trn_rl_repo")
if os.environ.get("JAX_PLATFORMS") not in (None, "", "axon"):
    os.environ["JAX_PLATFORMS"] = ""

import ml_dtypes
import numpy as np

import concourse.bass as bass
import concourse.tile as tile
from concourse import bacc, mybir
from concourse.bass_utils import run_bass_kernel_spmd
from concourse.masks import make_identity

F32 = mybir.dt.float32
F16 = mybir.dt.float16
BF16 = mybir.dt.bfloat16
FP8 = mybir.dt.float8e4
AF = mybir.ActivationFunctionType
OP = mybir.AluOpType
DR = mybir.MatmulPerfMode.DoubleRow

N, E, IN, H, C = 4096, 131072, 256, 4, 64
NCORES = 8
DL = N // NCORES          # 512 local rows per core
NB = N // 128             # 32 node blocks
MB = DL // 128            # 4 local blocks
F = H * C                 # 256
FX = 2 * F + 2 * H        # 520: [W_high | W_gat | wsrc | wdst] columns
T1C_INV = 0.5             # T1c stored as T1c/2 in fp8 (range margin)
PS_INV = 1.0 / 32.0       # applied to pt2 = P/2 -> Ps = P/64 in fp8
PS_SCALE = 64.0           # full P restore factor

_NC_CACHE = None


def _build_nc():
    nc = bacc.Bacc("TRN2", target_bir_lowering=False, debug=False,
                   num_devices=NCORES)
    t = lambda n, s, d: nc.dram_tensor(n, s, d, kind="ExternalInput").ap()
    xt8 = t("xt8", [IN, N], FP8)
    xtl8 = t("xtl8", [IN, DL], FP8)
    whg8 = t("whg8", [IN, FX], FP8)
    dinvt8 = t("dinvt8", [N, DL], FP8)
    lapt8 = t("lapt8", [N, DL], FP8)
    mlt8 = t("mlt8", [N, DL], BF16)
    biasb = t("biasb", [128, F], F32)
    avb = t("avb", [128, MB], F32)
    consts = t("consts", [128, 4], F32)
    out = nc.dram_tensor("out", [DL, F], F32, kind="ExternalOutput").ap()

    with tile.TileContext(nc) as tc:
        _emit(nc, tc, xt8=xt8, xtl8=xtl8, whg8=whg8, dinvt8=dinvt8,
              lapt8=lapt8, mlt8=mlt8, biasb=biasb, avb=avb,
              consts=consts, out=out)
    nc.compile()
    return nc


def _emit(nc, tc, *, xt8, xtl8, whg8, dinvt8, lapt8, mlt8, biasb, avb,
          consts, out):
    from contextlib import ExitStack
    ctx = ExitStack()
    with ctx:
        res = ctx.enter_context(tc.tile_pool(name="res", bufs=1))
        dr = ctx.enter_context(tc.tile_pool(name="dr", bufs=1, space="DRAM"))

        # ---------- resident tensors ----------
        h_sb = res.tile([128, NB * H * 65], BF16, name="h_sb")
        h4 = h_sb.rearrange("p (a b c) -> p a b c", a=NB, b=H)  # [128,32,4,65]
        xw_sb = res.tile([128, NB * F], FP8, name="xw_sb")
        xw3 = xw_sb.rearrange("p (a b) -> p a b", a=NB)         # [128,32,256]
        dinvt_sb = res.tile([128, NB * DL], FP8, name="dinvt_sb")
        di3 = dinvt_sb.rearrange("p (a b) -> p a b", a=NB)      # [128,32,512]
        lapt_sb = res.tile([128, NB * DL], FP8, name="lapt_sb")
        lp3 = lapt_sb.rearrange("p (a b) -> p a b", a=NB)
        t1g_sb = res.tile([128, NB * F], FP8, name="t1g_sb")
        t1g3 = t1g_sb.rearrange("p (a b) -> p a b", a=NB)
        psg_sb = res.tile([128, NB * F], FP8, name="psg_sb")
        psg3 = psg_sb.rearrange("p (a b) -> p a b", a=NB)
        mlt_sb = res.tile([128, NB * DL], BF16, name="mlt_sb")
        ml3 = mlt_sb.rearrange("p (a b) -> p a b", a=NB)        # [128,32,512]
        asrc_sb = res.tile([128, NB * H], F32, name="asrc_sb")
        adst_sb = res.tile([128, MB * H], F32, name="adst_sb")
        u1_sb = res.tile([128, NB * H], F32, name="u1_sb")
        u2_sb = res.tile([128, NB * H], F32, name="u2_sb")
        nu1_sb = res.tile([128, NB * H], F32, name="nu1_sb")
        wb_sb = res.tile([128, H * DL], BF16, name="wb_sb")
        wb3 = wb_sb.rearrange("p (a b) -> p a b", a=H)          # [128,4,512]
        wrow_sb = res.tile([1, H * DL], BF16, name="wrow_sb")
        wrow3 = wrow_sb.rearrange("p (a b) -> p a b", a=H)
        hl_sb = res.tile([128, MB * F], F32, name="hl_sb")
        gs_sb = res.tile([65, H * DL], BF16, name="gs_sb")
        gs3 = gs_sb.rearrange("p (a b) -> p a b", a=H)          # [65,4,512]
        bias_sb = res.tile([128, F], F32, name="bias_sb")
        avb_sb = res.tile([128, MB], F32, name="avb_sb")
        consts_sb = res.tile([128, 4], F32, name="consts_sb")
        cbc_sb = res.tile([128, F], F32, name="cbc_sb")
        crow_sb = res.tile([1, F], F32, name="crow_sb")
        bbc_sb = res.tile([128, F], F32, name="bbc_sb")
        brow_sb = res.tile([1, F], F32, name="brow_sb")
        t1c_sb = res.tile([128, MB * F], FP8, name="t1c_sb")
        ps_sb = res.tile([128, MB * F], FP8, name="ps_sb")
        ident = res.tile([128, 128], F32, name="ident")
        identb = res.tile([128, 128], BF16, name="identb")
        ones1 = res.tile([1, 128], F32, name="ones1")
        ones1b = res.tile([1, 128], BF16, name="ones1b")
        ones8 = res.tile([128, 2], FP8, name="ones8")
        o83 = ones8.rearrange("p (a b) -> p a b", a=2)          # [128,2,1]
        warm_sb = res.tile([1, 2], F16, name="warm_sb")

        # collective bounce buffers
        warm_in = dr.tile([1, 2], F16, name="warm_in")
        warm_out = dr.tile([NCORES, 2], F16, name="warm_out",
                           addr_space="Shared")
        t1_in = dr.tile([DL, F], FP8, name="t1_in")
        t1_out = dr.tile([N, F], FP8, name="t1_out", addr_space="Shared")
        t2_in = dr.tile([DL, F], FP8, name="t2_in")
        t2_out = dr.tile([N, F], FP8, name="t2_out", addr_space="Shared")

        # prologue-only
        pres = tc.alloc_tile_pool(name="pres", bufs=1)
        xt_sb = pres.tile([128, 2 * N], FP8, name="xt_sb")
        xt3 = xt_sb.rearrange("p (a b) -> p a b", a=2)          # [128,2,4096]
        xtl_sb = pres.tile([128, 2 * DL], FP8, name="xtl_sb")
        xtl3 = xtl_sb.rearrange("p (a b) -> p a b", a=2)
        whg_sb = pres.tile([128, 2 * FX], FP8, name="whg_sb")
        whg3 = whg_sb.rearrange("p (a b) -> p a b", a=2)        # [128,2,520]

        # ---------- warmup collective (absorbs launch skew) ----------
        nc.vector.memset(warm_sb[:], 0.0)
        nc.sync.dma_start(warm_in[:, :], warm_sb[:])
        nc.gpsimd.collective_compute(
            "AllGather", OP.bypass,
            replica_groups=[list(range(NCORES))],
            ins=[warm_in[:, :]], outs=[warm_out[:, :]])

        # ---------- constant loads ----------
        nc.sync.dma_start(xtl_sb[:], xtl8.rearrange("(a b) c -> b a c", a=2))
        nc.sync.dma_start(whg_sb[:], whg8.rearrange("(a b) c -> b a c", a=2))
        nc.sync.dma_start(consts_sb[:], consts[:, :])
        nc.sync.dma_start(xt_sb[:], xt8.rearrange("(a b) c -> b a c", a=2))
        nc.sync.dma_start(bias_sb[:], biasb[:, :])
        nc.sync.dma_start(avb_sb[:], avb[:, :])
        nc.sync.dma_start(dinvt_sb[:],
                          dinvt8.rearrange("(a b) c -> b a c", a=NB))
        nc.sync.dma_start(lapt_sb[:],
                          lapt8.rearrange("(a b) c -> b a c", a=NB))
        nc.sync.dma_start(mlt_sb[:],
                          mlt8.rearrange("(a b) c -> b a c", a=NB))
        make_identity(nc, ident[:])
        make_identity(nc, identb[:])
        nc.vector.memset(ones1[:], 1.0)
        nc.vector.memset(ones1b[:], 1.0)
        nc.vector.memset(ones8[:], 1.0)
        nc.vector.memset(h4[:, :, :, 64:65], 1.0)  # ones column of h_aug

        # GAT accumulators (live until finalize)
        gps = tc.alloc_tile_pool(name="gps", bufs=1, space="PSUM")
        g_t = [gps.tile([65, DL], F32, tag=f"g{h}", name=f"g_{h}")
               for h in range(H)]

        # ---------- P2: local-row a_dst (aux weight columns only) -----
        with tc.tile_pool(name="p2ps", bufs=2, space="PSUM") as p2ps:
            for mb in range(MB):
                ps2 = p2ps.tile([128, 2 * H], F32, tag="ps2",
                                name=f"ps2_{mb}")
                nc.tensor.matmul(ps2[:], xtl3[:, :, mb * 128:(mb + 1) * 128],
                                 whg3[:, :, 2 * F:FX], start=True, stop=True,
                                 perf_mode=DR, skip_group_check=True)
                nc.vector.tensor_scalar_add(
                    adst_sb[:, mb * H:(mb + 1) * H],
                    ps2[:, H:2 * H], 0.0)

        # ---------- P3: w = exp(-0.8 a_dst), broadcast to [128, DL] ----
        with tc.tile_pool(name="bcps", bufs=1, space="PSUM") as bcps:
            for h in range(H):
                pst = bcps.tile([1, DL], F32, tag="pst", name=f"pst_{h}")
                for mb in range(MB):
                    nc.tensor.transpose(
                        pst[0:1, mb * 128:(mb + 1) * 128],
                        adst_sb[:, mb * H + h:mb * H + h + 1], ident[:])
                nc.scalar.activation(wrow3[0:1, h, :], pst[0:1, :],
                                     AF.Exp, scale=-0.8)
                pwb = bcps.tile([128, DL], F32, tag="pwb", bufs=2,
                                name=f"pwb_{h}")
                nc.tensor.matmul(pwb[:], ones1b[:], wrow3[0:1, h, :],
                                 start=True, stop=True, skip_group_check=True)
                nc.scalar.copy(wb3[:, h, :], pwb[:])

        # ---------- P1: full XW (fp8 DoubleRow), h ----------
        with tc.tile_pool(name="pps", bufs=3, space="PSUM") as pps:
            for nb in range(NB):
                psx = pps.tile([128, 2 * F], F32, tag="psx", name=f"psx_{nb}")
                nc.tensor.matmul(psx[:], xt3[:, :, nb * 128:(nb + 1) * 128],
                                 whg3[:, :, 0:2 * F], start=True, stop=True,
                                 perf_mode=DR, skip_group_check=True)
                nc.scalar.activation(xw3[:, nb, :], psx[:, 0:F], AF.Relu)
                nc.scalar.copy(
                    h4[:, nb, :, 0:64],
                    psx[:, F:2 * F].rearrange("p (a b) -> p a b", a=H))

        # ---------- P1b: a_src columns ----------
        with tc.tile_pool(name="pas", bufs=4, space="PSUM") as pas:
            for nb in range(NB):
                psa = pas.tile([128, 2 * H], F32, tag="psa", name=f"psa_{nb}")
                nc.tensor.matmul(psa[:], xt3[:, :, nb * 128:(nb + 1) * 128],
                                 whg3[:, :, 2 * F:FX], start=True, stop=True,
                                 perf_mode=DR, skip_group_check=True)
                nc.vector.tensor_scalar_add(
                    asrc_sb[:, nb * H:(nb + 1) * H], psa[:, 0:H], 0.0)

        # u terms from a_src
        nc.scalar.activation(u1_sb[:], asrc_sb[:], AF.Exp)
        nc.scalar.activation(u2_sb[:], asrc_sb[:], AF.Exp, scale=0.2)
        nc.vector.tensor_scalar_mul(nu1_sb[:], u1_sb[:], -1.0)

        # ---------- T1c = Dc @ XW0 (fp8 DR), + c = 0.5*colsum(XW0) ----
        with tc.tile_pool(name="t1ps", bufs=2, space="PSUM") as t1ps, \
             tc.tile_pool(name="mups", bufs=1, space="PSUM") as mups:
            for m in range(MB):
                pt1 = t1ps.tile([128, F], F32, tag="pt1", name=f"pt1_{m}")
                for kp in range(NB // 2):
                    nc.tensor.matmul(
                        pt1[:],
                        di3[:, 2 * kp:2 * kp + 2, m * 128:(m + 1) * 128],
                        xw3[:, 2 * kp:2 * kp + 2, :],
                        start=(kp == 0), stop=(kp == NB // 2 - 1),
                        perf_mode=DR, skip_group_check=True)
                nc.scalar.activation(t1c_sb[:, m * F:(m + 1) * F], pt1[:],
                                     AF.Copy, scale=T1C_INV)
                nc.sync.dma_start(t1_in[m * 128:(m + 1) * 128, :],
                                  t1c_sb[:, m * F:(m + 1) * F])
            pmu = mups.tile([1, F], F32, tag="pmu", name="pmu")
            for k in range(NB):
                nc.tensor.matmul(pmu[0:1, :], ones8[:, 0:1],
                                 xw3[:, k, :],
                                 start=(k == 0), stop=(k == NB - 1),
                                 skip_group_check=True)
            nc.vector.tensor_scalar_mul(crow_sb[0:1, :], pmu[0:1, :], 0.5)
            pcb = mups.tile([128, F], F32, tag="pcb", name="pcb")
            nc.tensor.matmul(pcb[:], ones1[:], crow_sb[0:1, :],
                             start=True, stop=True, skip_group_check=True)
            nc.scalar.copy(cbc_sb[:], pcb[:])

        # ---------- AllGather T1c ----------
        nc.gpsimd.collective_compute(
            "AllGather", OP.bypass,
            replica_groups=[list(range(NCORES))],
            ins=[t1_in[:, :]], outs=[t1_out[:, :]])

        # ---------- GAT main loop + T2 in the middle ----------
        ep = tc.alloc_tile_pool(name="ep", bufs=3)

        def gat_block(sb):
            # r = relu(w*u2 - u1) on ACT;  pm = (r + u1) * mlt on DVE
            r_t = ep.tile([128, H * DL], BF16, tag="r", bufs=3,
                          name=f"r_{sb}")
            r3 = r_t.rearrange("p (a b) -> p a b", a=H)
            pm_t = ep.tile([128, H * DL], BF16, tag="pm", bufs=3,
                           name=f"pm_{sb}")
            pm3 = pm_t.rearrange("p (a b) -> p a b", a=H)
            for h in range(H):
                col = sb * H + h
                nc.scalar.activation(r3[:, h, :], wb3[:, h, :], AF.Relu,
                                     scale=u2_sb[:, col:col + 1],
                                     bias=nu1_sb[:, col:col + 1])
            for h in range(H):
                col = sb * H + h
                nc.vector.scalar_tensor_tensor(
                    pm3[:, h, :], r3[:, h, :], u1_sb[:, col:col + 1],
                    ml3[:, sb, :], op0=OP.add, op1=OP.mult)
            for h in range(H):
                nc.tensor.matmul(g_t[h][0:65, :], h4[:, sb, h, :],
                                 pm3[:, h, :], start=(sb == 0),
                                 stop=(sb == NB - 1), skip_group_check=True)

        for sb in range(20):
            gat_block(sb)

        nc.sync.dma_start(t1g_sb[:],
                          t1_out.rearrange("(a b) c -> b a c", a=NB))

        # ---------- T2: P/2 = lap @ T1g (fp8 DR), Ps = P/64 ----------
        t2ps = tc.alloc_tile_pool(name="t2ps", bufs=1, space="PSUM")
        pt2s = []
        for m in range(MB):
            pt2 = t2ps.tile([128, F], F32, tag=f"pt2_{m}", name=f"pt2_{m}")
            pt2s.append(pt2)
            for kp in range(NB // 2):
                nc.tensor.matmul(
                    pt2[:], lp3[:, 2 * kp:2 * kp + 2, m * 128:(m + 1) * 128],
                    t1g3[:, 2 * kp:2 * kp + 2, :],
                    start=(kp == 0), stop=(kp == NB // 2 - 1),
                    perf_mode=DR, skip_group_check=True)

        for sb in range(20, NB):
            gat_block(sb)

        # gs copies free the g banks; Ps casts on ACT after the GAT tail
        for h in range(H):
            nc.scalar.copy(gs3[:, h, :], g_t[h][0:65, :])
        for m in range(MB):
            nc.scalar.activation(ps_sb[:, m * F:(m + 1) * F], pt2s[m][:],
                                 AF.Copy, scale=PS_INV)
            nc.sync.dma_start(t2_in[m * 128:(m + 1) * 128, :],
                              ps_sb[:, m * F:(m + 1) * F])
        t2ps.release()
        ep.release()

        # ---------- AllGather Ps ----------
        nc.gpsimd.collective_compute(
            "AllGather", OP.bypass,
            replica_groups=[list(range(NCORES))],
            ins=[t2_in[:, :]], outs=[t2_out[:, :]])
        nc.sync.dma_start(psg_sb[:],
                          t2_out.rearrange("(a b) c -> b a c", a=NB))

        # ---------- GAT finalize: transpose, normalize, scale, bias ----
        with tc.tile_pool(name="trps", bufs=2, space="PSUM") as trps, \
             tc.tile_pool(name="gtp", bufs=4) as gtp, \
             tc.tile_pool(name="smalls", bufs=8) as smalls:
            for mb in range(MB):
                for h in range(H):
                    ptr = trps.tile([128, 128], BF16, tag="ptr")
                    nc.tensor.transpose(
                        ptr[0:128, 0:65],
                        gs3[:, h, mb * 128:(mb + 1) * 128],
                        identb[0:65, 0:65])
                    gt = gtp.tile([128, 65], F32, tag="gt")
                    nc.scalar.copy(gt[:], ptr[0:128, 0:65])
                    r = smalls.tile([128, 1], F32, tag="r")
                    nc.vector.reciprocal(r[:], gt[:, 64:65])
                    rs = smalls.tile([128, 1], F32, tag="rs")
                    nc.vector.tensor_scalar_mul(rs[:], r[:],
                                                consts_sb[:, 0:1])
                    nc.vector.scalar_tensor_tensor(
                        hl_sb[:, mb * F + h * C:mb * F + (h + 1) * C],
                        gt[:, 0:64], rs[:],
                        bias_sb[:, h * C:(h + 1) * C],
                        op0=OP.mult, op1=OP.add)
        gps.release()
        pres.release()

        # ---------- beta:  bbc = 0.5*PS_SCALE*aH*colsum(Ps) ------
        with tc.tile_pool(name="bps", bufs=1, space="PSUM") as bps:
            pb = bps.tile([1, F], F32, tag="pb", name="pb")
            for k in range(NB):
                nc.tensor.matmul(pb[0:1, :], ones8[:, 0:1],
                                 psg3[:, k, :],
                                 start=(k == 0), stop=(k == NB - 1),
                                 skip_group_check=True)
            nc.scalar.activation(brow_sb[0:1, :], pb[0:1, :], AF.Copy,
                                 scale=consts_sb[0:1, 2:3])
            pbb = bps.tile([128, F], F32, tag="pbb", name="pbb")
            nc.tensor.matmul(pbb[:], ones1[:], brow_sb[0:1, :],
                             start=True, stop=True, skip_group_check=True)
            nc.scalar.copy(bbc_sb[:], pbb[:])

            # ---------- T3 = Dc @ Ps (fp8 DR) + combine ----------
            with tc.tile_pool(name="t3ps", bufs=2, space="PSUM") as t3ps, \
                 tc.tile_pool(name="outp", bufs=4) as outp:
                for m in range(MB):
                    pt3 = t3ps.tile([128, F], F32, tag="pt3")
                    for kp in range(NB // 2):
                        nc.tensor.matmul(
                            pt3[:],
                            di3[:, 2 * kp:2 * kp + 2, m * 128:(m + 1) * 128],
                            psg3[:, 2 * kp:2 * kp + 2, :],
                            start=(kp == 0), stop=(kp == NB // 2 - 1),
                            perf_mode=DR, skip_group_check=True)
                    o1 = outp.tile([128, F], F32, tag="o1")
                    nc.vector.scalar_tensor_tensor(
                        o1[:], pt3[:], consts_sb[:, 1:2],
                        hl_sb[:, m * F:(m + 1) * F], op0=OP.mult, op1=OP.add)
                    o2 = outp.tile([128, F], F32, tag="o2")
                    nc.vector.scalar_tensor_tensor(
                        o2[:], cbc_sb[:], avb_sb[:, m:m + 1], o1[:],
                        op0=OP.mult, op1=OP.add)
                    o3 = outp.tile([128, F], F32, tag="o3")
                    nc.vector.tensor_tensor(o3[:], o2[:], bbc_sb[:],
                                            op=OP.add)
                    nc.sync.dma_start(out[m * 128:(m + 1) * 128, :], o3[:])


def _prep_inputs(x, edge_index, lap, d_inv, W_high, W_gat, att_src, att_dst,
                 bias_gat, aL, aH):
    f8 = ml_dtypes.float8_e4m3fn
    x = np.asarray(x, np.float32)
    edge_index = np.asarray(edge_index, np.int64)
    lap = np.asarray(lap, np.float32)
    d_inv = np.asarray(d_inv, np.float32)
    W_high = np.asarray(W_high, np.float32)
    W_gat = np.asarray(W_gat, np.float32)
    att_src = np.asarray(att_src, np.float32)
    att_dst = np.asarray(att_dst, np.float32)
    bias_gat = np.asarray(bias_gat, np.float32)
    aL = float(np.asarray(aL)); aH = float(np.asarray(aH))

    # edge multiplicity matrix [src, dst] + self loops
    M = np.zeros((N, N), np.float32)
    np.add.at(M, (edge_index[0], edge_index[1]), 1.0)
    M[np.arange(N), np.arange(N)] += 1.0

    # a_src/a_dst as extra weight columns: wsrc[h] = att_src[h] @ W_gat[h]
    Wg3 = W_gat.reshape(H, C, IN)
    wsrc = np.einsum('hc,hci->hi', att_src, Wg3)       # [H, IN]
    wdst = np.einsum('hc,hci->hi', att_dst, Wg3)       # [H, IN]
    whg = np.concatenate([W_high.T, W_gat.T, wsrc.T, wdst.T], axis=1)

    # rank-1 helpers (input-only preprocessing)
    lsum = lap.sum(axis=1)                              # [N]
    v = d_inv @ lsum                                    # [N]

    xt8 = np.ascontiguousarray(x.T).astype(f8)
    whg8 = np.ascontiguousarray(whg).astype(f8)
    bias_b = np.broadcast_to(bias_gat, (128, F)).astype(np.float32)
    consts_b = np.broadcast_to(
        np.array([aL, PS_SCALE * aH, 0.5 * PS_SCALE * aH, 0.0],
                 np.float32), (128, 4))

    in_maps = []
    for c in range(NCORES):
        rows = slice(c * DL, (c + 1) * DL)
        av = (aH * v[rows]).reshape(MB, 128).T
        in_maps.append({
            "xt8": xt8,
            "xtl8": np.ascontiguousarray(x[rows].T).astype(f8),
            "whg8": whg8,
            "dinvt8": np.ascontiguousarray(
                (d_inv[rows] - 0.5).T).astype(f8),
            "lapt8": np.ascontiguousarray(lap[rows].T).astype(f8),
            "mlt8": np.ascontiguousarray(M[:, rows]).astype(
                ml_dtypes.bfloat16),
            "biasb": np.ascontiguousarray(bias_b),
            "avb": np.ascontiguousarray(av.astype(np.float32)),
            "consts": np.ascontiguousarray(consts_b),
        })
    return in_maps


def kernel(x, edge_index, lap, d_inv, W_high, W_gat, att_src, att_dst,
           bias_gat, aL, aH):
    global _NC_CACHE
    if _NC_CACHE is None:
        _NC_CACHE = _build_nc()
    nc = _NC_CACHE
    in_maps = _prep_inputs(x, edge_index, lap, d_inv, W_high, W_gat,
                           att_src, att_dst, bias_gat, aL, aH)
    trace = bool(int(os.environ.get("BASS_TRACE_KERNEL", "0")))
    res = run_bass_kernel_spmd(nc, in_maps, core_ids=list(range(NCORES)),
                               trace=trace)
    kernel.last_exec_time_ns = res.exec_time_ns
    kernel.last_results = res
    return np.concatenate([res.results[c]["out"] for c in range(NCORES)],
                          axis=0).astype(np.float32)


kernel.last_exec_time_ns = None
kernel.last_results = None


# revision 36
# speedup vs baseline: 1.4472x; 1.4472x over previous
"""FBGAT layer kernel for 8 Trainium2 NeuronCores (v2: fp8 DoubleRow).

Full inputs in, full output out. Row-shards nodes across 8 cores.

High-pass path Hh = d_inv @ lap @ d_inv @ relu(X Wh^T).  With the host
centering d_inv = 0.5*J + Dc and host-precomputed (input-only)
lsum = lap @ 1, v = d_inv @ lsum, and device c = 0.5*colsum(XW0):
    T1 = d_inv @ XW0 = 1 (x) c + T1c,   T1c = Dc @ XW0
    T2 = lap @ T1    = lsum (x) c + P,  P   = lap @ T1c
    Hh = d_inv @ T2  = v (x) c + 1 (x) beta + Dc @ P,
                                        beta = 0.5*colsum(P)
T1c / Ps = P/16 ship between cores as fp8 AllGathers; every chain
matmul is fp8 DoubleRow (two 128-k-planes per instruction).  Rank-1
terms are rebuilt in the output combine.  Total error ~1e-3 of output
absmax (tolerance 2e-2).

GAT: dense [src, dst] form.  Per-dst softmax rescale by exp(-a_dst)
turns exp(leakyrelu(a_src+a_dst)) into max(u1[s], u2[s]*w[d]) with
u1 = exp(a_src), u2 = exp(0.2 a_src), w = exp(-0.8 a_dst) -- the
rescale cancels between numerator and denominator.  So the dense part
is one relu/max pass plus one multiply by the edge-multiplicity
matrix; no dense exp.  Numerator and denominator both come from one PE
matmul with a ones-augmented h.  a_src / a_dst come out of the main
XW matmul via 8 extra weight columns folded host-side.
"""
import os
import sys

sys.path.insert(0, "/opt/trn_rl_repo")
if os.environ.get("JAX_PLATFORMS") not in (None, "", "axon"):
    os.environ["JAX_PLATFORMS"] = ""

import ml_dtypes
import numpy as np

import concourse.bass as bass
import concourse.tile as tile
from concourse import bacc, mybir
from concourse.bass_utils import run_bass_kernel_spmd
from concourse.masks import make_identity

F32 = mybir.dt.float32
F16 = mybir.dt.float16
BF16 = mybir.dt.bfloat16
FP8 = mybir.dt.float8e4
AF = mybir.ActivationFunctionType
OP = mybir.AluOpType
DR = mybir.MatmulPerfMode.DoubleRow

N, E, IN, H, C = 4096, 131072, 256, 4, 64
NCORES = 8
DL = N // NCORES          # 512 local rows per core
NB = N // 128             # 32 node blocks
MB = DL // 128            # 4 local blocks
F = H * C                 # 256
FX = 2 * F + 2 * H        # 520: [W_high | W_gat | wsrc | wdst] columns
T1C_INV = 0.5             # T1c stored as T1c/2 in fp8 (range margin)
PS_INV = 1.0 / 32.0       # applied to pt2 = P/2 -> Ps = P/64 in fp8
PS_SCALE = 64.0           # full P restore factor

_NC_CACHE = None


def _build_nc():
    nc = bacc.Bacc("TRN2", target_bir_lowering=False, debug=False,
                   num_devices=NCORES)
    t = lambda n, s, d: nc.dram_tensor(n, s, d, kind="ExternalInput").ap()
    xt8 = t("xt8", [IN, N], FP8)
    xtl8 = t("xtl8", [IN, DL], FP8)
    whg8 = t("whg8", [IN, FX], FP8)
    dinvt8 = t("dinvt8", [N, DL], FP8)
    lapt8 = t("lapt8", [N, DL], FP8)
    mlt8 = t("mlt8", [N, DL], BF16)
    biasb = t("biasb", [128, F], F32)
    avb = t("avb", [128, MB], F32)
    consts = t("consts", [128, 4], F32)
    out = nc.dram_tensor("out", [DL, F], F32, kind="ExternalOutput").ap()

    with tile.TileContext(nc) as tc:
        _emit(nc, tc, xt8=xt8, xtl8=xtl8, whg8=whg8, dinvt8=dinvt8,
              lapt8=lapt8, mlt8=mlt8, biasb=biasb, avb=avb,
              consts=consts, out=out)
    nc.compile()
    return nc


def _emit(nc, tc, *, xt8, xtl8, whg8, dinvt8, lapt8, mlt8, biasb, avb,
          consts, out):
    from contextlib import ExitStack
    ctx = ExitStack()
    with ctx:
        res = ctx.enter_context(tc.tile_pool(name="res", bufs=1))
        dr = ctx.enter_context(tc.tile_pool(name="dr", bufs=1, space="DRAM"))

        # ---------- resident tensors ----------
        h_sb = res.tile([128, NB * H * 65], BF16, name="h_sb")
        h4 = h_sb.rearrange("p (a b c) -> p a b c", a=NB, b=H)  # [128,32,4,65]
        xw_sb = res.tile([128, NB * F], FP8, name="xw_sb")
        xw3 = xw_sb.rearrange("p (a b) -> p a b", a=NB)         # [128,32,256]
        dinvt_sb = res.tile([128, NB * DL], FP8, name="dinvt_sb")
        di3 = dinvt_sb.rearrange("p (a b) -> p a b", a=NB)      # [128,32,512]
        lapt_sb = res.tile([128, NB * DL], FP8, name="lapt_sb")
        lp3 = lapt_sb.rearrange("p (a b) -> p a b", a=NB)
        t1g_sb = res.tile([128, NB * F], FP8, name="t1g_sb")
        t1g3 = t1g_sb.rearrange("p (a b) -> p a b", a=NB)
        psg_sb = res.tile([128, NB * F], FP8, name="psg_sb")
        psg3 = psg_sb.rearrange("p (a b) -> p a b", a=NB)
        asrc_sb = res.tile([128, NB * H], F32, name="asrc_sb")
        adst_sb = res.tile([128, MB * H], F32, name="adst_sb")
        u1_sb = res.tile([128, NB * H], F32, name="u1_sb")
        u2_sb = res.tile([128, NB * H], F32, name="u2_sb")
        nu1_sb = res.tile([128, NB * H], F32, name="nu1_sb")
        wb_sb = res.tile([128, H * DL], BF16, name="wb_sb")
        wb3 = wb_sb.rearrange("p (a b) -> p a b", a=H)          # [128,4,512]
        wrow_sb = res.tile([1, H * DL], BF16, name="wrow_sb")
        wrow3 = wrow_sb.rearrange("p (a b) -> p a b", a=H)
        hl_sb = res.tile([128, MB * F], F32, name="hl_sb")
        gs_sb = res.tile([65, H * DL], BF16, name="gs_sb")
        gs3 = gs_sb.rearrange("p (a b) -> p a b", a=H)          # [65,4,512]
        bias_sb = res.tile([128, F], F32, name="bias_sb")
        avb_sb = res.tile([128, MB], F32, name="avb_sb")
        consts_sb = res.tile([128, 4], F32, name="consts_sb")
        cbc_sb = res.tile([128, F], F32, name="cbc_sb")
        crow_sb = res.tile([1, F], F32, name="crow_sb")
        bbc_sb = res.tile([128, F], F32, name="bbc_sb")
        brow_sb = res.tile([1, F], F32, name="brow_sb")
        t1c_sb = res.tile([128, MB * F], FP8, name="t1c_sb")
        ps_sb = res.tile([128, MB * F], FP8, name="ps_sb")
        ident = res.tile([128, 128], F32, name="ident")
        identb = res.tile([128, 128], BF16, name="identb")
        ones1 = res.tile([1, 128], F32, name="ones1")
        ones1b = res.tile([1, 128], BF16, name="ones1b")
        ones8 = res.tile([128, 2], FP8, name="ones8")
        o83 = ones8.rearrange("p (a b) -> p a b", a=2)          # [128,2,1]
        warm_sb = res.tile([1, 2], F16, name="warm_sb")

        # collective bounce buffers
        warm_in = dr.tile([1, 2], F16, name="warm_in")
        warm_out = dr.tile([NCORES, 2], F16, name="warm_out",
                           addr_space="Shared")
        t1_in = dr.tile([DL, F], FP8, name="t1_in")
        t1_out = dr.tile([N, F], FP8, name="t1_out", addr_space="Shared")
        t2_in = dr.tile([DL, F], FP8, name="t2_in")
        t2_out = dr.tile([N, F], FP8, name="t2_out", addr_space="Shared")

        # prologue-only
        pres = tc.alloc_tile_pool(name="pres", bufs=1)
        xt_sb = pres.tile([128, 2 * N], FP8, name="xt_sb")
        xt3 = xt_sb.rearrange("p (a b) -> p a b", a=2)          # [128,2,4096]
        xtl_sb = pres.tile([128, 2 * DL], FP8, name="xtl_sb")
        xtl3 = xtl_sb.rearrange("p (a b) -> p a b", a=2)
        whg_sb = pres.tile([128, 2 * FX], FP8, name="whg_sb")
        whg3 = whg_sb.rearrange("p (a b) -> p a b", a=2)        # [128,2,520]

        # ---------- warmup collective (absorbs launch skew) ----------
        nc.vector.memset(warm_sb[:], 0.0)
        nc.sync.dma_start(warm_in[:, :], warm_sb[:])
        nc.gpsimd.collective_compute(
            "AllGather", OP.bypass,
            replica_groups=[list(range(NCORES))],
            ins=[warm_in[:, :]], outs=[warm_out[:, :]])

        # ---------- constant loads ----------
        nc.sync.dma_start(xtl_sb[:], xtl8.rearrange("(a b) c -> b a c", a=2))
        nc.sync.dma_start(whg_sb[:], whg8.rearrange("(a b) c -> b a c", a=2))
        nc.sync.dma_start(consts_sb[:], consts[:, :])
        nc.sync.dma_start(xt_sb[:], xt8.rearrange("(a b) c -> b a c", a=2))
        nc.sync.dma_start(bias_sb[:], biasb[:, :])
        nc.sync.dma_start(avb_sb[:], avb[:, :])
        nc.sync.dma_start(dinvt_sb[:],
                          dinvt8.rearrange("(a b) c -> b a c", a=NB))
        nc.sync.dma_start(lapt_sb[:],
                          lapt8.rearrange("(a b) c -> b a c", a=NB))
        make_identity(nc, ident[:])
        make_identity(nc, identb[:])
        nc.vector.memset(ones1[:], 1.0)
        nc.vector.memset(ones1b[:], 1.0)
        nc.vector.memset(ones8[:], 1.0)
        nc.vector.memset(h4[:, :, :, 64:65], 1.0)  # ones column of h_aug

        # GAT accumulators (live until finalize)
        gps = tc.alloc_tile_pool(name="gps", bufs=1, space="PSUM")
        g_t = [gps.tile([65, DL], F32, tag=f"g{h}", name=f"g_{h}")
               for h in range(H)]

        # ---------- P2: local-row a_dst (aux weight columns only) -----
        with tc.tile_pool(name="p2ps", bufs=2, space="PSUM") as p2ps:
            for mb in range(MB):
                ps2 = p2ps.tile([128, 2 * H], F32, tag="ps2",
                                name=f"ps2_{mb}")
                nc.tensor.matmul(ps2[:], xtl3[:, :, mb * 128:(mb + 1) * 128],
                                 whg3[:, :, 2 * F:FX], start=True, stop=True,
                                 perf_mode=DR, skip_group_check=True)
                nc.vector.tensor_scalar_add(
                    adst_sb[:, mb * H:(mb + 1) * H],
                    ps2[:, H:2 * H], 0.0)

        # ---------- P3: w = exp(-0.8 a_dst), broadcast to [128, DL] ----
        with tc.tile_pool(name="bcps", bufs=1, space="PSUM") as bcps:
            for h in range(H):
                pst = bcps.tile([1, DL], F32, tag="pst", name=f"pst_{h}")
                for mb in range(MB):
                    nc.tensor.transpose(
                        pst[0:1, mb * 128:(mb + 1) * 128],
                        adst_sb[:, mb * H + h:mb * H + h + 1], ident[:])
                nc.scalar.activation(wrow3[0:1, h, :], pst[0:1, :],
                                     AF.Exp, scale=-0.8)
                pwb = bcps.tile([128, DL], F32, tag="pwb", bufs=2,
                                name=f"pwb_{h}")
                nc.tensor.matmul(pwb[:], ones1b[:], wrow3[0:1, h, :],
                                 start=True, stop=True, skip_group_check=True)
                nc.scalar.copy(wb3[:, h, :], pwb[:])

        # ---------- P1: full XW (fp8 DoubleRow), h + a_src ----------
        with tc.tile_pool(name="pps", bufs=2, space="PSUM") as pps:
            for nb in range(NB):
                psx = pps.tile([128, 2 * F], F32, tag="psx", name=f"psx_{nb}")
                nc.tensor.matmul(psx[:], xt3[:, :, nb * 128:(nb + 1) * 128],
                                 whg3[:, :, 0:2 * F], start=True, stop=True,
                                 perf_mode=DR, skip_group_check=True)
                psa = pps.tile([128, 2 * H], F32, tag="psa", name=f"psa_{nb}")
                nc.tensor.matmul(psa[:], xt3[:, :, nb * 128:(nb + 1) * 128],
                                 whg3[:, :, 2 * F:FX], start=True, stop=True,
                                 perf_mode=DR, skip_group_check=True)
                nc.scalar.activation(xw3[:, nb, :], psx[:, 0:F], AF.Relu)
                nc.vector.tensor_copy(
                    h4[:, nb, :, 0:64],
                    psx[:, F:2 * F].rearrange("p (a b) -> p a b", a=H))
                nc.vector.tensor_scalar_add(
                    asrc_sb[:, nb * H:(nb + 1) * H], psa[:, 0:H], 0.0)
                if nb in (15, NB - 1):
                    # u terms for the finished half so GAT can start early
                    lo = 0 if nb == 15 else 64
                    hi = 64 if nb == 15 else 128
                    nc.scalar.activation(u1_sb[:, lo:hi],
                                         asrc_sb[:, lo:hi], AF.Exp)
                    nc.scalar.activation(u2_sb[:, lo:hi],
                                         asrc_sb[:, lo:hi], AF.Exp,
                                         scale=0.2)
                    nc.vector.tensor_scalar_mul(nu1_sb[:, lo:hi],
                                                u1_sb[:, lo:hi], -1.0)

        # ---------- T1c = Dc @ XW0 (fp8 DR), + c = 0.5*colsum(XW0) ----
        with tc.tile_pool(name="t1ps", bufs=2, space="PSUM") as t1ps, \
             tc.tile_pool(name="mups", bufs=1, space="PSUM") as mups:
            for m in range(MB):
                pt1 = t1ps.tile([128, F], F32, tag="pt1", name=f"pt1_{m}")
                for kp in range(NB // 2):
                    nc.tensor.matmul(
                        pt1[:],
                        di3[:, 2 * kp:2 * kp + 2, m * 128:(m + 1) * 128],
                        xw3[:, 2 * kp:2 * kp + 2, :],
                        start=(kp == 0), stop=(kp == NB // 2 - 1),
                        perf_mode=DR, skip_group_check=True)
                nc.scalar.activation(t1c_sb[:, m * F:(m + 1) * F], pt1[:],
                                     AF.Copy, scale=T1C_INV)
                nc.sync.dma_start(t1_in[m * 128:(m + 1) * 128, :],
                                  t1c_sb[:, m * F:(m + 1) * F])
            pmu = mups.tile([1, F], F32, tag="pmu", name="pmu")
            for k in range(NB):
                nc.tensor.matmul(pmu[0:1, :], ones8[:, 0:1],
                                 xw3[:, k, :],
                                 start=(k == 0), stop=(k == NB - 1),
                                 skip_group_check=True)
            nc.vector.tensor_scalar_mul(crow_sb[0:1, :], pmu[0:1, :], 0.5)
            pcb = mups.tile([128, F], F32, tag="pcb", name="pcb")
            nc.tensor.matmul(pcb[:], ones1[:], crow_sb[0:1, :],
                             start=True, stop=True, skip_group_check=True)
            nc.scalar.copy(cbc_sb[:], pcb[:])

        # ---------- AllGather T1c ----------
        nc.gpsimd.collective_compute(
            "AllGather", OP.bypass,
            replica_groups=[list(range(NCORES))],
            ins=[t1_in[:, :]], outs=[t1_out[:, :]])

        # ---------- GAT main loop + T2 in the middle ----------
        mltp = tc.alloc_tile_pool(name="mltp", bufs=3)
        ep = tc.alloc_tile_pool(name="ep", bufs=3)

        def gat_block(sb):
            mlt_t = mltp.tile([128, DL], BF16, tag="mlt_t", name=f"mlt_{sb}")
            nc.sync.dma_start(mlt_t[:], mlt8[sb * 128:(sb + 1) * 128, :])
            pm_t = ep.tile([128, H * DL], BF16, tag="pm", bufs=3,
                           name=f"pm_{sb}")
            pm3 = pm_t.rearrange("p (a b) -> p a b", a=H)
            if sb % 3 != 1:
                # ACT path: r = relu(w*u2 - u1); pm = (r + u1) * mlt
                r_t = ep.tile([128, H * DL], BF16, tag="r", bufs=3,
                              name=f"r_{sb}")
                r3 = r_t.rearrange("p (a b) -> p a b", a=H)
                for h in range(H):
                    col = sb * H + h
                    nc.scalar.activation(r3[:, h, :], wb3[:, h, :], AF.Relu,
                                         scale=u2_sb[:, col:col + 1],
                                         bias=nu1_sb[:, col:col + 1])
                for h in range(H):
                    col = sb * H + h
                    nc.vector.scalar_tensor_tensor(
                        pm3[:, h, :], r3[:, h, :], u1_sb[:, col:col + 1],
                        mlt_t[:], op0=OP.add, op1=OP.mult)
            else:
                # DVE path: t = max(w*u2, u1); pm = t * mlt
                t_t = ep.tile([128, H * DL], BF16, tag="r", bufs=3,
                              name=f"t_{sb}")
                t3 = t_t.rearrange("p (a b) -> p a b", a=H)
                for h in range(H):
                    col = sb * H + h
                    nc.vector.tensor_scalar(
                        t3[:, h, :], wb3[:, h, :], u2_sb[:, col:col + 1],
                        u1_sb[:, col:col + 1], op0=OP.mult, op1=OP.max)
                mbc = bass.AP(mlt_t.tensor, mlt_t.offset,
                              [mlt_t.ap[0], [0, H], [1, DL]])
                nc.vector.tensor_tensor(pm_t[:], t_t[:], mbc, op=OP.mult)
            for h in range(H):
                nc.tensor.matmul(g_t[h][0:65, :], h4[:, sb, h, :],
                                 pm3[:, h, :], start=(sb == 0),
                                 stop=(sb == NB - 1), skip_group_check=True)

        for sb in range(8):
            gat_block(sb)

        # t1g load sits in the sync queue after gat 0-7's mlt loads
        nc.sync.dma_start(t1g_sb[:],
                          t1_out.rearrange("(a b) c -> b a c", a=NB))

        # ---------- T2: P = lap @ T1g (fp8 DR), Ps = P/16 ----------
        t2ps = tc.alloc_tile_pool(name="t2ps", bufs=1, space="PSUM")
        pt2s = []

        def t2_mm(m):
            pt2 = t2ps.tile([128, F], F32, tag=f"pt2_{m}", name=f"pt2_{m}")
            pt2s.append(pt2)
            for kp in range(NB // 2):
                nc.tensor.matmul(
                    pt2[:], lp3[:, 2 * kp:2 * kp + 2, m * 128:(m + 1) * 128],
                    t1g3[:, 2 * kp:2 * kp + 2, :],
                    start=(kp == 0), stop=(kp == NB // 2 - 1),
                    perf_mode=DR, skip_group_check=True)

        def t2_fin(m):
            nc.vector.tensor_scalar_mul(ps_sb[:, m * F:(m + 1) * F],
                                        pt2s[m][:], PS_INV)
            nc.sync.dma_start(t2_in[m * 128:(m + 1) * 128, :],
                              ps_sb[:, m * F:(m + 1) * F])

        t2_mm(0)
        t2_mm(1)
        for sb in range(8, 10):
            gat_block(sb)
        t2_mm(2)
        t2_mm(3)
        for sb in range(10, 12):
            gat_block(sb)
        for m in range(MB):
            t2_fin(m)
        t2ps.release()

        # ---------- AllGather Ps ----------
        nc.gpsimd.collective_compute(
            "AllGather", OP.bypass,
            replica_groups=[list(range(NCORES))],
            ins=[t2_in[:, :]], outs=[t2_out[:, :]])

        for sb in range(12, NB):
            gat_block(sb)

        # psg load after the gat mlt loads so it can't stall them
        nc.sync.dma_start(psg_sb[:],
                          t2_out.rearrange("(a b) c -> b a c", a=NB))
        ep.release()
        mltp.release()

        # ---------- GAT finalize: transpose, normalize, scale, bias ----
        for h in range(H):
            nc.scalar.copy(gs3[:, h, :], g_t[h][0:65, :])
        with tc.tile_pool(name="trps", bufs=2, space="PSUM") as trps, \
             tc.tile_pool(name="gtp", bufs=2) as gtp, \
             tc.tile_pool(name="smalls", bufs=4) as smalls:
            for mb in range(MB):
                ptr = trps.tile([128, H * 66], BF16, tag="ptr")
                for h in range(H):
                    nc.tensor.transpose(
                        ptr[0:128, h * 66:h * 66 + 65],
                        gs3[:, h, mb * 128:(mb + 1) * 128],
                        identb[0:65, 0:65])
                gt = gtp.tile([128, H * 66], F32, tag="gt")
                nc.scalar.copy(gt[:], ptr[:])
                gt3 = gt.rearrange("p (a b) -> p a b", a=H)
                r = smalls.tile([128, H], F32, tag="r")
                nc.vector.reciprocal(r[:], gt3[:, :, 64:65])
                rs = smalls.tile([128, H], F32, tag="rs")
                nc.vector.tensor_scalar_mul(rs[:], r[:],
                                            consts_sb[:, 0:1])
                for h in range(H):
                    nc.vector.scalar_tensor_tensor(
                        hl_sb[:, mb * F + h * C:mb * F + (h + 1) * C],
                        gt3[:, h, 0:64], rs[:, h:h + 1],
                        bias_sb[:, h * C:(h + 1) * C],
                        op0=OP.mult, op1=OP.add)
        gps.release()
        pres.release()

        # ---------- beta = 0.5*colsum(P);  bbc = 8*aH*colsum(Ps) ------
        with tc.tile_pool(name="bps", bufs=1, space="PSUM") as bps:
            pb = bps.tile([1, F], F32, tag="pb", name="pb")
            for k in range(NB):
                nc.tensor.matmul(pb[0:1, :], ones8[:, 0:1],
                                 psg3[:, k, :],
                                 start=(k == 0), stop=(k == NB - 1),
                                 skip_group_check=True)
            nc.scalar.activation(brow_sb[0:1, :], pb[0:1, :], AF.Copy,
                                 scale=consts_sb[0:1, 2:3])
            pbb = bps.tile([128, F], F32, tag="pbb", name="pbb")
            nc.tensor.matmul(pbb[:], ones1[:], brow_sb[0:1, :],
                             start=True, stop=True, skip_group_check=True)
            nc.scalar.copy(bbc_sb[:], pbb[:])

            # ---------- T3 = Dc @ Ps (fp8 DR) + combine ----------
            with tc.tile_pool(name="t3ps", bufs=2, space="PSUM") as t3ps, \
                 tc.tile_pool(name="outp", bufs=4) as outp:
                for m in range(MB):
                    pt3 = t3ps.tile([128, F], F32, tag="pt3")
                    for kp in range(NB // 2):
                        nc.tensor.matmul(
                            pt3[:],
                            di3[:, 2 * kp:2 * kp + 2, m * 128:(m + 1) * 128],
                            psg3[:, 2 * kp:2 * kp + 2, :],
                            start=(kp == 0), stop=(kp == NB // 2 - 1),
                            perf_mode=DR, skip_group_check=True)
                    o1 = outp.tile([128, F], F32, tag="o1")
                    nc.vector.scalar_tensor_tensor(
                        o1[:], pt3[:], consts_sb[:, 1:2],
                        hl_sb[:, m * F:(m + 1) * F], op0=OP.mult, op1=OP.add)
                    o2 = outp.tile([128, F], F32, tag="o2")
                    nc.vector.scalar_tensor_tensor(
                        o2[:], cbc_sb[:], avb_sb[:, m:m + 1], o1[:],
                        op0=OP.mult, op1=OP.add)
                    o3 = outp.tile([128, F], F32, tag="o3")
                    nc.vector.tensor_tensor(o3[:], o2[:], bbc_sb[:],
                                            op=OP.add)
                    nc.sync.dma_start(out[m * 128:(m + 1) * 128, :], o3[:])


def _prep_inputs(x, edge_index, lap, d_inv, W_high, W_gat, att_src, att_dst,
                 bias_gat, aL, aH):
    f8 = ml_dtypes.float8_e4m3fn
    x = np.asarray(x, np.float32)
    edge_index = np.asarray(edge_index, np.int64)
    lap = np.asarray(lap, np.float32)
    d_inv = np.asarray(d_inv, np.float32)
    W_high = np.asarray(W_high, np.float32)
    W_gat = np.asarray(W_gat, np.float32)
    att_src = np.asarray(att_src, np.float32)
    att_dst = np.asarray(att_dst, np.float32)
    bias_gat = np.asarray(bias_gat, np.float32)
    aL = float(np.asarray(aL)); aH = float(np.asarray(aH))

    # edge multiplicity matrix [src, dst] + self loops
    M = np.zeros((N, N), np.float32)
    np.add.at(M, (edge_index[0], edge_index[1]), 1.0)
    M[np.arange(N), np.arange(N)] += 1.0

    # a_src/a_dst as extra weight columns: wsrc[h] = att_src[h] @ W_gat[h]
    Wg3 = W_gat.reshape(H, C, IN)
    wsrc = np.einsum('hc,hci->hi', att_src, Wg3)       # [H, IN]
    wdst = np.einsum('hc,hci->hi', att_dst, Wg3)       # [H, IN]
    whg = np.concatenate([W_high.T, W_gat.T, wsrc.T, wdst.T], axis=1)

    # rank-1 helpers (input-only preprocessing)
    lsum = lap.sum(axis=1)                              # [N]
    v = d_inv @ lsum                                    # [N]

    xt8 = np.ascontiguousarray(x.T).astype(f8)
    whg8 = np.ascontiguousarray(whg).astype(f8)
    bias_b = np.broadcast_to(bias_gat, (128, F)).astype(np.float32)
    consts_b = np.broadcast_to(
        np.array([aL, PS_SCALE * aH, 0.5 * PS_SCALE * aH, 0.0],
                 np.float32), (128, 4))

    in_maps = []
    for c in range(NCORES):
        rows = slice(c * DL, (c + 1) * DL)
        av = (aH * v[rows]).reshape(MB, 128).T
        in_maps.append({
            "xt8": xt8,
            "xtl8": np.ascontiguousarray(x[rows].T).astype(f8),
            "whg8": whg8,
            "dinvt8": np.ascontiguousarray(
                (d_inv[rows] - 0.5).T).astype(f8),
            "lapt8": np.ascontiguousarray(lap[rows].T).astype(f8),
            "mlt8": np.ascontiguousarray(M[:, rows]).astype(
                ml_dtypes.bfloat16),
            "biasb": np.ascontiguousarray(bias_b),
            "avb": np.ascontiguousarray(av.astype(np.float32)),
            "consts": np.ascontiguousarray(consts_b),
        })
    return in_maps


def kernel(x, edge_index, lap, d_inv, W_high, W_gat, att_src, att_dst,
           bias_gat, aL, aH):
    global _NC_CACHE
    if _NC_CACHE is None:
        _NC_CACHE = _build_nc()
    nc = _NC_CACHE
    in_maps = _prep_inputs(x, edge_index, lap, d_inv, W_high, W_gat,
                           att_src, att_dst, bias_gat, aL, aH)
    trace = bool(int(os.environ.get("BASS_TRACE_KERNEL", "0")))
    res = run_bass_kernel_spmd(nc, in_maps, core_ids=list(range(NCORES)),
                               trace=trace)
    kernel.last_exec_time_ns = res.exec_time_ns
    kernel.last_results = res
    return np.concatenate([res.results[c]["out"] for c in range(NCORES)],
                          axis=0).astype(np.float32)


kernel.last_exec_time_ns = None
kernel.last_results = None
